# revision 9
# baseline (speedup 1.0000x reference)
"""Trainium2 Bass kernel for nn_MultiHeadAttention_89120571392727.

Reference computation (fp32):
    q = (Q @ Wq + bq)  -> [B, H, L, dk]   (per-head split)
    k, v likewise
    scores = q @ k^T / sqrt(dk);  scores = where(mask, -1e9, scores)
    attn = softmax(scores); context = attn @ v
    out = LayerNorm(Q + context_cat @ Wo + bo) * gamma + beta
    returns (out [B,L,D], attn [B,H,L,L])

Sharding (8 cores, no collectives): core c handles batch b=c//2 and
query-row half hf=c%2 (1024 rows), for ALL 16 heads.  Each core:
  - projections (bf16 matmuls, fp32 accumulate)
  - scores computed TRANSPOSED (keys on partitions, queries on free axis)
  - additive mask folded in via a PE identity-matmul accumulating an
    fp8 mask (0 / -57344) into the scores PSUM
  - exp on ScalarE with scale=1/8 (=1/sqrt(dk)); softmax denominators
    ride along the context matmul as a prepended ones-column of v
  - normalization on VectorE; attn written transposed [H, Lk, Lq]
    (host fixes the layout); out-projection + residual + LayerNorm on
    device.
Host work is limited to shard slicing / transposition / dtype casts of
inputs and reassembly of outputs.
"""

import os
import sys
from contextlib import ExitStack

import numpy as np

for _p in ("/opt/trn_rl_repo", "/opt/pypackages"):
    if _p not in sys.path and os.path.isdir(_p):
        sys.path.append(_p)

import ml_dtypes  # noqa: E402

BF16 = ml_dtypes.bfloat16
F8E5 = ml_dtypes.float8_e5m2
MASK_NEG = -57344.0  # most-negative finite f8e5m2; exp(-57344/8) == 0 in fp32

# Full-problem dimensions (hardcoded per the harness contract).
FULL_CFG = dict(
    D=1024,      # model dim
    H=16,        # heads
    DK=64,       # head dim
    LQ=1024,     # query rows per core (2048 / 2 halves)
    LK=2048,     # key rows
    QC=512,      # query-column chunk (<= 512, PSUM bank)
    KG=4,        # key-tile group size for probability staging
    EPS=1e-5,
)
N_CORES = 8
B_FULL, L_FULL, D_FULL, H_FULL = 4, 2048, 1024, 16

# Engine that casts fp32 probabilities to bf16 for the context matmul.
CAST_ENGINE = "gpsimd"  # or "vector"


def build_program(cfg, num_devices=N_CORES):
    """Build + schedule + compile the (SPMD, per-core) Bass program."""
    from concourse import bacc, mybir
    import concourse.tile as tile

    P = 128
    D, H, DK, LQ, LK, QC = (cfg[k] for k in ("D", "H", "DK", "LQ", "LK", "QC"))
    KG, EPS = cfg["KG"], cfg["EPS"]
    HD = H * DK
    DT, HDT, KT, LQT = D // P, HD // P, LK // P, LQ // P
    NQC = LQ // QC
    NG = KT // KG
    AW = DK + 1  # per-head width in v_aug (ones column first)
    assert D % P == 0 and HD % P == 0 and LK % P == 0 and LQ % P == 0
    assert QC <= 512 and LQ % QC == 0 and KT % KG == 0 and DK == 64

    f32 = mybir.dt.float32
    bf16 = mybir.dt.bfloat16
    f8 = mybir.dt.float8e5

    nc = bacc.Bacc("TRN2", target_bir_lowering=False, debug=False,
                   num_devices=num_devices)

    def din(name, shape, dtype):
        return nc.dram_tensor(name, list(shape), dtype, kind="ExternalInput").ap()

    def dout(name, shape, dtype):
        return nc.dram_tensor(name, list(shape), dtype, kind="ExternalOutput").ap()

    t = dict(
        QT_in=din("QT_in", (D, LQ), bf16),
        KT_in=din("KT_in", (D, LK), bf16),
        VT_in=din("VT_in", (D, LK), bf16),
        maskT=din("maskT", (LK, LQ), f8),
        ident=din("ident", (P, P), f8),
        Wq=din("Wq", (D, HD), bf16),
        Wk=din("Wk", (D, HD), bf16),
        Wv=din("Wv", (D, HD), bf16),
        Wo=din("Wo", (HD, D), bf16),
        bq=din("bq", (HD,), f32),
        bk=din("bk", (HD,), f32),
        bv=din("bv", (HD,), f32),
        bo=din("bo", (D,), f32),
        gamma=din("gamma", (D,), f32),
        beta=din("beta", (D,), f32),
        Qh=din("Qh", (LQ, D), f32),
        out_h=dout("out_h", (LQ, D), f32),
        attn_t=dout("attn_t", (H, LK, LQ), f32),
    )

    import concourse.bass as bass

    def bcast_from_dram(vec_ap, parts=P):
        """[N] dram vector -> broadcast-read AP [parts, N] (partition step 0)."""
        return bass.AP(tensor=vec_ap.tensor, offset=vec_ap.offset,
                       ap=[[0, parts]] + [list(d) for d in vec_ap.ap])

    with tile.TileContext(nc) as tc, ExitStack() as ctx:
        Copy = mybir.ActivationFunctionType.Copy
        Identity = mybir.ActivationFunctionType.Identity
        Exp = mybir.ActivationFunctionType.Exp
        Sqrt = mybir.ActivationFunctionType.Sqrt
        cast_engine = getattr(nc, CAST_ENGINE)

        persist = ctx.enter_context(tc.tile_pool(name="persist", bufs=1))
        qT = persist.tile([P, HDT, LQ], bf16, name="qT")
        kT = persist.tile([P, HDT, LK], bf16, name="kT")
        v_aug = persist.tile([P, KT, H * AW], bf16, name="v_aug")
        ctx_cat = persist.tile([P, HDT, LQ], bf16, name="ctx_cat")
        ident_sb = persist.tile([P, P], f8, name="ident_sb")
        nc.sync.dma_start(out=ident_sb, in_=t["ident"])

        # ---------------- stage 1: projections ----------------
        with ExitStack() as s1:
            xw = s1.enter_context(tc.tile_pool(name="xw", bufs=2))
            xin = s1.enter_context(tc.tile_pool(name="xin", bufs=1))
            bpool = s1.enter_context(tc.tile_pool(name="bias1", bufs=1))
            ps1 = s1.enter_context(tc.tile_pool(name="ps1", bufs=4, space="PSUM"))

            bq_sb = bpool.tile([P, HDT], f32, name="bq_sb")
            nc.sync.dma_start(out=bq_sb, in_=t["bq"].rearrange("(t p) -> p t", p=P))
            bk_sb = bpool.tile([P, HDT], f32, name="bk_sb")
            nc.sync.dma_start(out=bk_sb, in_=t["bk"].rearrange("(t p) -> p t", p=P))
            bv_sb = bpool.tile([P, HD], f32, name="bv_sb")
            nc.sync.dma_start(out=bv_sb, in_=bcast_from_dram(t["bv"]))

            # q / k projections produce transposed activations
            # xT_out[d', l] = sum_D W[D, d'] * XT_in[D, l]  (+ bias[d'])
            def proj_T(w_name, b_sb, in_name, out_tile, L):
                w_sb = xw.tile([P, DT, HD], bf16, tag="w")
                nc.sync.dma_start(
                    out=w_sb, in_=t[w_name].rearrange("(t p) n -> p t n", p=P))
                x_sb = xin.tile([P, DT, L], bf16, tag=f"xin{L}")
                nc.sync.dma_start(
                    out=x_sb, in_=t[in_name].rearrange("(t p) l -> p t l", p=P))
                CH = min(512, L)
                for ot in range(HDT):
                    for ci in range(L // CH):
                        ps = ps1.tile([P, CH], f32, tag="ps1")
                        for dt_ in range(DT):
                            nc.tensor.matmul(
                                ps,
                                lhsT=w_sb[:, dt_, ot * P:(ot + 1) * P],
                                rhs=x_sb[:, dt_, ci * CH:(ci + 1) * CH],
                                start=(dt_ == 0), stop=(dt_ == DT - 1))
                        nc.scalar.activation(
                            out=out_tile[:, ot, ci * CH:(ci + 1) * CH], in_=ps,
                            func=Identity, bias=b_sb[:, ot:ot + 1], scale=1.0)

            proj_T("Wq", bq_sb, "QT_in", qT, LQ)
            proj_T("Wk", bk_sb, "KT_in", kT, LK)

            # v projection in natural orientation: v[l, d'] (+ ones column
            # per head, prepended -> softmax denominators ride the matmul)
            w_sb = xw.tile([P, DT, HD], bf16, tag="w")
            nc.sync.dma_start(
                out=w_sb, in_=t["Wv"].rearrange("(t p) n -> p t n", p=P))
            x_sb = xin.tile([P, DT, LK], bf16, tag=f"xin{LK}")
            nc.sync.dma_start(
                out=x_sb, in_=t["VT_in"].rearrange("(t p) l -> p t l", p=P))
            v_by_head = v_aug.rearrange("p k (h w) -> p k h w", w=AW)
            nc.vector.memset(v_by_head[:, :, :, DK:DK + 1], 1.0)
            CHV = min(512, HD)
            HPC = CHV // DK  # heads per chunk
            for lt in range(KT):
                for ci in range(HD // CHV):
                    ps = ps1.tile([P, CHV], f32, tag="ps1")
                    for dt_ in range(DT):
                        nc.tensor.matmul(
                            ps,
                            lhsT=x_sb[:, dt_, lt * P:(lt + 1) * P],
                            rhs=w_sb[:, dt_, ci * CHV:(ci + 1) * CHV],
                            start=(dt_ == 0), stop=(dt_ == DT - 1))
                    nc.vector.tensor_add(
                        out=v_by_head[:, lt, ci * HPC:(ci + 1) * HPC, 0:DK],
                        in0=ps.rearrange("p (h d) -> p h d", d=DK),
                        in1=bv_sb.rearrange("p (h d) -> p h d", d=DK)[
                            :, ci * HPC:(ci + 1) * HPC, :])

        # ---------------- stage 2: attention ----------------
        with ExitStack() as s2:
            mpool = s2.enter_context(tc.tile_pool(name="maskp", bufs=2))
            ppool = s2.enter_context(
                tc.tile_pool(name="probs", bufs=max(NG + 3, 2 * NG - 1)))
            pbpool = s2.enter_context(tc.tile_pool(name="probs_bf", bufs=NG))
            rpool = s2.enter_context(tc.tile_pool(name="rvec", bufs=3))
            stpool = s2.enter_context(tc.tile_pool(name="ctxstage", bufs=2))
            ps_sc = s2.enter_context(tc.tile_pool(name="ps_sc", bufs=4, space="PSUM"))
            ps_cx = s2.enter_context(tc.tile_pool(name="ps_cx", bufs=2, space="PSUM"))
            ps_rb = s2.enter_context(tc.tile_pool(name="ps_rb", bufs=2, space="PSUM"))

            # ones row on partition DK (=64), matching the sums row of the
            # context PSUM so the K=1 broadcast matmul is row-consistent
            ones_c = rpool.tile([DK + 1, P], f32, name="ones_c", bufs=1)
            nc.vector.memset(ones_c[DK:DK + 1, :], 1.0)

            maskT_dram = t["maskT"].rearrange("(t p) q -> p t q", p=P)
            for qi in range(NQC):
                qsl = slice(qi * QC, (qi + 1) * QC)
                mT = mpool.tile([P, KT, QC], f8, tag="mT")
                nc.sync.dma_start(out=mT, in_=maskT_dram[:, :, qsl])
                for h in range(H):
                    po = (h % 2) * 64
                    ht = h // 2
                    ps_c = ps_cx.tile([AW, QC], f32, tag="psc")
                    p4s = []
                    for g in range(NG):
                        P4 = ppool.tile([P, KG, QC], f32, tag="p4")
                        Pb4 = pbpool.tile([P, KG, QC], bf16, tag="pb4")
                        for j in range(KG):
                            kt = g * KG + j
                            ps_s = ps_sc.tile([P, QC], f32, tag="pss")
                            nc.tensor.matmul(
                                ps_s,
                                lhsT=kT[po:po + DK, ht, kt * P:(kt + 1) * P],
                                rhs=qT[po:po + DK, ht, qsl],
                                start=True, stop=False)
                            nc.tensor.matmul(
                                ps_s, lhsT=ident_sb, rhs=mT[:, kt, :],
                                start=False, stop=True)
                            nc.scalar.activation(
                                out=P4[:, j, :], in_=ps_s, func=Exp, scale=0.125)
                            cast_engine.tensor_copy(out=Pb4[:, j, :], in_=P4[:, j, :])
                            nc.tensor.matmul(
                                ps_c, lhsT=v_aug[:, kt, h * AW:(h + 1) * AW],
                                rhs=Pb4[:, j, :],
                                start=(kt == 0), stop=(kt == KT - 1))
                        p4s.append(P4)
                    # softmax denominators (sums row = partition DK) ->
                    # reciprocal -> broadcast to all 128 partitions via a
                    # K=1 matmul against the ones row
                    r = rpool.tile([DK + 1, QC], f32, tag="r")
                    nc.vector.reciprocal(out=r[DK:DK + 1, :],
                                         in_=ps_c[DK:DK + 1, :])
                    rb = ps_rb.tile([P, QC], f32, tag="rb")
                    nc.tensor.matmul(rb, lhsT=ones_c[DK:DK + 1, :],
                                     rhs=r[DK:DK + 1, :], start=True, stop=True)
                    rb_sb = rpool.tile([P, QC], f32, tag="rb_sb")
                    nc.scalar.copy(out=rb_sb, in_=rb)
                    att_dst = t["attn_t"][h].rearrange("(t p) q -> p t q", p=P)
                    rb_b = rb_sb[:, None, :].to_broadcast((P, KG, QC))
                    for g, P4 in enumerate(p4s):
                        nc.vector.tensor_mul(out=P4, in0=P4, in1=rb_b)
                        nc.sync.dma_start(
                            out=att_dst[:, g * KG:(g + 1) * KG, qsl], in_=P4)
                    # context rows live on psum partitions 0..63; normalize
                    # into ctx_cat partitions po..po+63 (odd heads need a
                    # partition shift, which only DMA can do)
                    if po == 0:
                        nc.vector.tensor_mul(
                            out=ctx_cat[0:DK, ht, qsl],
                            in0=ps_c[0:DK, :], in1=rb_sb[0:DK, :])
                    else:
                        ctmp = stpool.tile([DK, QC], f32, tag="ctmp")
                        nc.scalar.activation(out=ctmp, in_=ps_c[0:DK, :],
                                             func=Copy)
                        stg = stpool.tile([P, QC], f32, tag="stg")
                        nc.sync.dma_start(out=stg[po:po + DK, :], in_=ctmp)
                        nc.vector.tensor_mul(
                            out=ctx_cat[po:po + DK, ht, qsl],
                            in0=stg[po:po + DK, :], in1=rb_sb[po:po + DK, :])

        # ---------------- stage 3: out-projection + LayerNorm ----------------
        with ExitStack() as s3:
            wpool = s3.enter_context(tc.tile_pool(name="wo_pool", bufs=1))
            opool = s3.enter_context(tc.tile_pool(name="outw", bufs=3))
            ps3 = s3.enter_context(tc.tile_pool(name="ps3", bufs=4, space="PSUM"))

            wo_sb = wpool.tile([P, HDT, D], bf16, name="wo_sb")
            nc.sync.dma_start(out=wo_sb,
                              in_=t["Wo"].rearrange("(t p) n -> p t n", p=P))
            qres = wpool.tile([P, LQT, D], f32, name="qres")
            nc.sync.dma_start(out=qres,
                              in_=t["Qh"].rearrange("(t p) d -> p t d", p=P))
            bo_b = wpool.tile([P, D], f32, name="bo_b")
            nc.sync.dma_start(out=bo_b, in_=bcast_from_dram(t["bo"]))
            gam_b = wpool.tile([P, D], f32, name="gam_b")
            nc.sync.dma_start(out=gam_b, in_=bcast_from_dram(t["gamma"]))
            bet_b = wpool.tile([P, D], f32, name="bet_b")
            nc.sync.dma_start(out=bet_b, in_=bcast_from_dram(t["beta"]))
            eps_t = wpool.tile([P, 1], f32, name="eps_t")
            nc.vector.memset(eps_t, EPS)

            CHO = min(512, D)
            SG = D // CHO  # bn_stats subgroups
            out_dst = t["out_h"].rearrange("(t p) d -> p t d", p=P)
            for lt in range(LQT):
                s_t = opool.tile([P, D], f32, tag="s_t")
                for ci in range(D // CHO):
                    ps = ps3.tile([P, CHO], f32, tag="pso")
                    for dt_ in range(HDT):
                        nc.tensor.matmul(
                            ps,
                            lhsT=ctx_cat[:, dt_, lt * P:(lt + 1) * P],
                            rhs=wo_sb[:, dt_, ci * CHO:(ci + 1) * CHO],
                            start=(dt_ == 0), stop=(dt_ == HDT - 1))
                        # residual + bias
                    csl = slice(ci * CHO, (ci + 1) * CHO)
                    nc.vector.tensor_add(out=s_t[:, csl], in0=ps,
                                         in1=qres[:, lt, csl])
                nc.vector.tensor_add(out=s_t, in0=s_t, in1=bo_b)
                stats = opool.tile([P, SG, 6], f32, tag="stats")
                for i in range(SG):
                    nc.vector.bn_stats(out=stats[:, i, :],
                                       in_=s_t[:, i * CHO:(i + 1) * CHO])
                mv = opool.tile([P, 2], f32, tag="mv")
                nc.vector.bn_aggr(out=mv, in_=stats)
                std = opool.tile([P, 1], f32, tag="std")
                nc.scalar.activation(out=std, in_=mv[:, 1:2], func=Sqrt,
                                     bias=eps_t, scale=1.0)
                rstd = opool.tile([P, 1], f32, tag="rstd")
                nc.vector.reciprocal(out=rstd, in_=std)
                o_t = opool.tile([P, D], f32, tag="o_t")
                nc.vector.tensor_scalar(
                    out=o_t, in0=s_t, scalar1=mv[:, 0:1], scalar2=rstd,
                    op0=mybir.AluOpType.subtract, op1=mybir.AluOpType.mult)
                nc.vector.tensor_mul(out=o_t, in0=o_t, in1=gam_b)
                nc.vector.tensor_add(out=o_t, in0=o_t, in1=bet_b)
                nc.sync.dma_start(out=out_dst[:, lt, :], in_=o_t)

    nc.compile()
    return nc


def make_core_inputs(cfg, Q, K, V, attn_mask, Wq, bq, Wk, bk, Wv, bv, Wo, bo,
                     gamma, beta):
    """Host-side shard prep. Returns list of 8 input dicts (core order)."""
    P = 128
    LQ = cfg["LQ"]
    w_b = {n: np.asarray(w, np.float32).astype(BF16)
           for n, w in (("Wq", Wq), ("Wk", Wk), ("Wv", Wv), ("Wo", Wo))}
    vecs = {n: np.ascontiguousarray(np.asarray(v, np.float32))
            for n, v in (("bq", bq), ("bk", bk), ("bv", bv), ("bo", bo),
                         ("gamma", gamma), ("beta", beta))}
    ident = np.eye(P, dtype=np.float32).astype(F8E5)
    B = Q.shape[0]
    per_b = {}
    for b in range(B):
        per_b[b] = dict(
            KT_in=np.ascontiguousarray(
                np.asarray(K[b], np.float32).T).astype(BF16),
            VT_in=np.ascontiguousarray(
                np.asarray(V[b], np.float32).T).astype(BF16),
        )
    in_maps = []
    n_halves = Q.shape[1] // LQ
    for c in range(B * n_halves):
        b, hf = divmod(c, n_halves)
        qs = slice(hf * LQ, (hf + 1) * LQ)
        Qh = np.ascontiguousarray(np.asarray(Q[b, qs], np.float32))
        QT_in = np.ascontiguousarray(Qh.T).astype(BF16)
        m = np.asarray(attn_mask[b, qs], bool)
        maskT = np.where(m.T, np.float32(MASK_NEG),
                         np.float32(0.0)).astype(F8E5)
        in_maps.append(dict(
            QT_in=QT_in, maskT=maskT, ident=ident, Qh=Qh,
            **per_b[b], **w_b, **vecs))
    return in_maps


_PROGRAM_CACHE = {}


def _get_program():
    key = "full"
    if key not in _PROGRAM_CACHE:
        _PROGRAM_CACHE[key] = build_program(FULL_CFG)
    return _PROGRAM_CACHE[key]


def run_on_hw(inputs, trace=False, **kw):
    """Run the full-size kernel on the 8 NeuronCores. Returns BassKernelResults."""
    from concourse.bass_utils import run_bass_kernel_spmd
    nc = _get_program()
    in_maps = make_core_inputs(FULL_CFG, **inputs)
    return run_bass_kernel_spmd(nc, in_maps, core_ids=list(range(N_CORES)),
                                trace=trace, **kw)


def kernel(Q, K, V, attn_mask, Wq, bq, Wk, bk, Wv, bv, Wo, bo, gamma, beta):
    inputs = dict(Q=np.asarray(Q), K=np.asarray(K), V=np.asarray(V),
                  attn_mask=np.asarray(attn_mask), Wq=Wq, bq=bq, Wk=Wk, bk=bk,
                  Wv=Wv, bv=bv, Wo=Wo, bo=bo, gamma=gamma, beta=beta)
    res = run_on_hw(inputs).results
    B, L, D, H = B_FULL, L_FULL, D_FULL, H_FULL
    LQ = FULL_CFG["LQ"]
    out = np.empty((B, L, D), np.float32)
    attn = np.empty((B, H, L, L), np.float32)
    n_halves = L // LQ
    for c in range(N_CORES):
        b, hf = divmod(c, n_halves)
        qs = slice(hf * LQ, (hf + 1) * LQ)
        out[b, qs] = res[c]["out_h"]
        attn[b, :, qs, :] = res[c]["attn_t"].transpose(0, 2, 1)
    return out, attn


# revision 13
# speedup vs baseline: 1.2634x; 1.2634x over previous
"""Trainium2 Bass kernel for nn_MultiHeadAttention_89120571392727.

Reference computation (fp32):
    q = (Q @ Wq + bq)  -> [B, H, L, dk]   (per-head split)
    k, v likewise
    scores = q @ k^T / sqrt(dk);  scores = where(mask, -1e9, scores)
    attn = softmax(scores); context = attn @ v
    out = LayerNorm(Q + context_cat @ Wo + bo) * gamma + beta
    returns (out [B,L,D], attn [B,H,L,L])

Sharding (8 cores, no collectives): core c handles batch b=c//2 and
query-row half hf=c%2 (1024 rows), for ALL 16 heads.  Each core:
  - projections (bf16 matmuls, fp32 accumulate)
  - scores computed TRANSPOSED (keys on partitions, queries on free axis)
  - additive mask folded in via a PE identity-matmul accumulating an
    fp8 mask (0 / -57344) into the scores PSUM
  - exp on ScalarE with scale=1/8 (=1/sqrt(dk)); softmax denominators
    ride along the context matmul as a prepended ones-column of v
  - normalization on VectorE; attn written transposed [H, Lk, Lq]
    (host fixes the layout); out-projection + residual + LayerNorm on
    device.
Host work is limited to shard slicing / transposition / dtype casts of
inputs and reassembly of outputs.
"""

import os
import sys
from contextlib import ExitStack

import numpy as np

for _p in ("/opt/trn_rl_repo", "/opt/pypackages"):
    if _p not in sys.path and os.path.isdir(_p):
        sys.path.append(_p)

import ml_dtypes  # noqa: E402

F16 = np.float16
F8E5 = ml_dtypes.float8_e5m2
MASK_NEG = -57344.0  # most-negative finite f8e5m2; exp(-57344/8) == 0 in fp32

# Full-problem dimensions (hardcoded per the harness contract).
FULL_CFG = dict(
    D=1024,      # model dim
    H=16,        # heads
    DK=64,       # head dim
    LQ=1024,     # query rows per core (2048 / 2 halves)
    LK=2048,     # key rows
    QC=512,      # query-column chunk (<= 512, PSUM bank)
    KG=4,        # key-tile group size for probability staging
    EPS=1e-5,
)
N_CORES = 8
B_FULL, L_FULL, D_FULL, H_FULL = 4, 2048, 1024, 16

# Engine that casts fp32 probabilities to bf16 for the context matmul.
CAST_ENGINE = "gpsimd"  # or "vector"


def build_program(cfg, num_devices=N_CORES):
    """Build + schedule + compile the (SPMD, per-core) Bass program."""
    from concourse import bacc, mybir
    import concourse.tile as tile

    P = 128
    D, H, DK, LQ, LK, QC = (cfg[k] for k in ("D", "H", "DK", "LQ", "LK", "QC"))
    KG, EPS = cfg["KG"], cfg["EPS"]
    HD = H * DK
    DT, HDT, KT, LQT = D // P, HD // P, LK // P, LQ // P
    NQC = LQ // QC
    NG = KT // KG
    AW = DK + 1  # per-head width in v_aug (ones column first)
    assert D % P == 0 and HD % P == 0 and LK % P == 0 and LQ % P == 0
    assert QC <= 512 and LQ % QC == 0 and KT % KG == 0 and DK == 64

    f32 = mybir.dt.float32
    f16 = mybir.dt.float16
    f8 = mybir.dt.float8e5

    nc = bacc.Bacc("TRN2", target_bir_lowering=False, debug=False,
                   num_devices=num_devices)

    def din(name, shape, dtype):
        return nc.dram_tensor(name, list(shape), dtype, kind="ExternalInput").ap()

    def dout(name, shape, dtype):
        return nc.dram_tensor(name, list(shape), dtype, kind="ExternalOutput").ap()

    t = dict(
        QT_in=din("QT_in", (D, LQ), f16),
        KT_in=din("KT_in", (D, LK), f16),
        VT_in=din("VT_in", (D, LK), f16),
        maskT=din("maskT", (LK, LQ), f8),
        ident=din("ident", (P, P), f8),
        Wq=din("Wq", (D, HD), f16),
        Wk=din("Wk", (D, HD), f16),
        Wv=din("Wv", (D, HD), f16),
        Wo=din("Wo", (HD, D), f16),
        bq=din("bq", (HD,), f32),
        bk=din("bk", (HD,), f32),
        bv=din("bv", (HD,), f32),
        bo=din("bo", (D,), f32),
        gamma=din("gamma", (D,), f32),
        beta=din("beta", (D,), f32),
        Qh=din("Qh", (LQ, D), f32),
        out_h=dout("out_h", (LQ, D), f32),
        attn_t=dout("attn_t", (H, LK, LQ), f32),
    )

    import concourse.bass as bass

    def bcast_from_dram(vec_ap, parts=P):
        """[N] dram vector -> broadcast-read AP [parts, N] (partition step 0)."""
        return bass.AP(tensor=vec_ap.tensor, offset=vec_ap.offset,
                       ap=[[0, parts]] + [list(d) for d in vec_ap.ap])

    with tile.TileContext(nc) as tc, ExitStack() as ctx:
        Copy = mybir.ActivationFunctionType.Copy
        Identity = mybir.ActivationFunctionType.Identity
        Exp = mybir.ActivationFunctionType.Exp
        Sqrt = mybir.ActivationFunctionType.Sqrt

        persist = ctx.enter_context(tc.tile_pool(name="persist", bufs=1))
        qT = persist.tile([P, HDT, LQ], f16, name="qT")
        kT = persist.tile([P, HDT, LK], f16, name="kT")
        v_aug = persist.tile([P, KT, H * AW], f16, name="v_aug")
        ctx_cat = persist.tile([P, HDT, LQ], f16, name="ctx_cat")
        ident_sb = persist.tile([P, P], f8, name="ident_sb")
        nc.sync.dma_start(out=ident_sb, in_=t["ident"])

        # ---------------- stage 1: projections ----------------
        with ExitStack() as s1:
            xw = s1.enter_context(tc.tile_pool(name="xw", bufs=2))
            xin = s1.enter_context(tc.tile_pool(name="xin", bufs=1))
            bpool = s1.enter_context(tc.tile_pool(name="bias1", bufs=1))
            ps1 = s1.enter_context(tc.tile_pool(name="ps1", bufs=4, space="PSUM"))

            bq_sb = bpool.tile([P, HDT], f32, name="bq_sb")
            nc.sync.dma_start(out=bq_sb, in_=t["bq"].rearrange("(t p) -> p t", p=P))
            bk_sb = bpool.tile([P, HDT], f32, name="bk_sb")
            nc.sync.dma_start(out=bk_sb, in_=t["bk"].rearrange("(t p) -> p t", p=P))
            bv_sb = bpool.tile([P, HD], f32, name="bv_sb")
            nc.sync.dma_start(out=bv_sb, in_=bcast_from_dram(t["bv"]))

            # q / k projections produce transposed activations
            # xT_out[d', l] = sum_D W[D, d'] * XT_in[D, l]  (+ bias[d'])
            def proj_T(w_name, b_sb, in_name, out_tile, L):
                w_sb = xw.tile([P, DT, HD], f16, tag="w")
                nc.sync.dma_start(
                    out=w_sb, in_=t[w_name].rearrange("(t p) n -> p t n", p=P))
                x_sb = xin.tile([P, DT, L], f16, tag=f"xin{L}")
                nc.sync.dma_start(
                    out=x_sb, in_=t[in_name].rearrange("(t p) l -> p t l", p=P))
                CH = min(512, L)
                for ot in range(HDT):
                    for ci in range(L // CH):
                        ps = ps1.tile([P, CH], f32, tag="ps1")
                        for dt_ in range(DT):
                            nc.tensor.matmul(
                                ps,
                                lhsT=w_sb[:, dt_, ot * P:(ot + 1) * P],
                                rhs=x_sb[:, dt_, ci * CH:(ci + 1) * CH],
                                start=(dt_ == 0), stop=(dt_ == DT - 1))
                        nc.scalar.activation(
                            out=out_tile[:, ot, ci * CH:(ci + 1) * CH], in_=ps,
                            func=Identity, bias=b_sb[:, ot:ot + 1], scale=1.0)

            proj_T("Wq", bq_sb, "QT_in", qT, LQ)
            proj_T("Wk", bk_sb, "KT_in", kT, LK)

            # v projection in natural orientation: v[l, d'] (+ ones column
            # per head, prepended -> softmax denominators ride the matmul)
            w_sb = xw.tile([P, DT, HD], f16, tag="w")
            nc.sync.dma_start(
                out=w_sb, in_=t["Wv"].rearrange("(t p) n -> p t n", p=P))
            x_sb = xin.tile([P, DT, LK], f16, tag=f"xin{LK}")
            nc.sync.dma_start(
                out=x_sb, in_=t["VT_in"].rearrange("(t p) l -> p t l", p=P))
            v_by_head = v_aug.rearrange("p k (h w) -> p k h w", w=AW)
            nc.vector.memset(v_by_head[:, :, :, DK:DK + 1], 1.0)
            CHV = min(512, HD)
            HPC = CHV // DK  # heads per chunk
            for lt in range(KT):
                for ci in range(HD // CHV):
                    ps = ps1.tile([P, CHV], f32, tag="ps1")
                    for dt_ in range(DT):
                        nc.tensor.matmul(
                            ps,
                            lhsT=x_sb[:, dt_, lt * P:(lt + 1) * P],
                            rhs=w_sb[:, dt_, ci * CHV:(ci + 1) * CHV],
                            start=(dt_ == 0), stop=(dt_ == DT - 1))
                    nc.vector.tensor_add(
                        out=v_by_head[:, lt, ci * HPC:(ci + 1) * HPC, 0:DK],
                        in0=ps.rearrange("p (h d) -> p h d", d=DK),
                        in1=bv_sb.rearrange("p (h d) -> p h d", d=DK)[
                            :, ci * HPC:(ci + 1) * HPC, :])

        # ---------------- stage 2: attention ----------------
        with ExitStack() as s2:
            mpool = s2.enter_context(tc.tile_pool(name="maskp", bufs=2))
            ppool = s2.enter_context(
                tc.tile_pool(name="probs", bufs=max(NG + 3, 2 * NG + 2)))
            a4pool = s2.enter_context(tc.tile_pool(name="attn_st", bufs=4))
            rpool = s2.enter_context(tc.tile_pool(name="rvec", bufs=3))
            stpool = s2.enter_context(tc.tile_pool(name="ctxstage", bufs=2))
            ps_sc = s2.enter_context(tc.tile_pool(name="ps_sc", bufs=4, space="PSUM"))
            ps_cx = s2.enter_context(tc.tile_pool(name="ps_cx", bufs=2, space="PSUM"))
            ps_rb = s2.enter_context(tc.tile_pool(name="ps_rb", bufs=2, space="PSUM"))

            # ones row on partition DK (=64), matching the sums row of the
            # context PSUM so the K=1 broadcast matmul is row-consistent
            ones_c = rpool.tile([DK + 1, P], f32, name="ones_c", bufs=1)
            nc.vector.memset(ones_c[DK:DK + 1, :], 1.0)

            maskT_dram = t["maskT"].rearrange("(t p) q -> p t q", p=P)
            for qi in range(NQC):
                qsl = slice(qi * QC, (qi + 1) * QC)
                mT = mpool.tile([P, KT, QC], f8, tag="mT")
                nc.sync.dma_start(out=mT, in_=maskT_dram[:, :, qsl])
                for h in range(H):
                    po = (h % 2) * 64
                    ht = h // 2
                    ps_c = ps_cx.tile([AW, QC], f32, tag="psc")
                    p4s = []
                    for g in range(NG):
                        P4 = ppool.tile([P, KG, QC], f16, tag="p4")
                        for j in range(KG):
                            kt = g * KG + j
                            ps_s = ps_sc.tile([P, QC], f32, tag="pss")
                            nc.tensor.matmul(
                                ps_s,
                                lhsT=kT[po:po + DK, ht, kt * P:(kt + 1) * P],
                                rhs=qT[po:po + DK, ht, qsl],
                                start=True, stop=False)
                            nc.tensor.matmul(
                                ps_s, lhsT=ident_sb, rhs=mT[:, kt, :],
                                start=False, stop=True)
                            nc.scalar.activation(
                                out=P4[:, j, :], in_=ps_s, func=Exp, scale=0.125)
                            nc.tensor.matmul(
                                ps_c, lhsT=v_aug[:, kt, h * AW:(h + 1) * AW],
                                rhs=P4[:, j, :],
                                start=(kt == 0), stop=(kt == KT - 1))
                        p4s.append(P4)
                    # softmax denominators (sums row = partition DK) ->
                    # reciprocal -> broadcast to all 128 partitions via a
                    # K=1 matmul against the ones row
                    r = rpool.tile([DK + 1, QC], f32, tag="r")
                    nc.vector.reciprocal(out=r[DK:DK + 1, :],
                                         in_=ps_c[DK:DK + 1, :])
                    rb = ps_rb.tile([P, QC], f32, tag="rb")
                    nc.tensor.matmul(rb, lhsT=ones_c[DK:DK + 1, :],
                                     rhs=r[DK:DK + 1, :], start=True, stop=True)
                    rb_sb = rpool.tile([P, QC], f32, tag="rb_sb")
                    nc.scalar.copy(out=rb_sb, in_=rb)
                    att_dst = t["attn_t"][h].rearrange("(t p) q -> p t q", p=P)
                    rb_b = rb_sb[:, None, :].to_broadcast((P, KG, QC))
                    for g, P4 in enumerate(p4s):
                        A4 = a4pool.tile([P, KG, QC], f32, tag="a4")
                        nc.vector.tensor_mul(out=A4, in0=P4, in1=rb_b)
                        nc.sync.dma_start(
                            out=att_dst[:, g * KG:(g + 1) * KG, qsl], in_=A4)
                    # context rows live on psum partitions 0..63; normalize
                    # into ctx_cat partitions po..po+63 (odd heads need a
                    # partition shift, which only DMA can do)
                    if po == 0:
                        nc.vector.tensor_mul(
                            out=ctx_cat[0:DK, ht, qsl],
                            in0=ps_c[0:DK, :], in1=rb_sb[0:DK, :])
                    else:
                        ctmp = stpool.tile([DK, QC], f32, tag="ctmp")
                        nc.scalar.activation(out=ctmp, in_=ps_c[0:DK, :],
                                             func=Copy)
                        stg = stpool.tile([P, QC], f32, tag="stg")
                        nc.sync.dma_start(out=stg[po:po + DK, :], in_=ctmp)
                        nc.vector.tensor_mul(
                            out=ctx_cat[po:po + DK, ht, qsl],
                            in0=stg[po:po + DK, :], in1=rb_sb[po:po + DK, :])

        # ---------------- stage 3: out-projection + LayerNorm ----------------
        with ExitStack() as s3:
            wpool = s3.enter_context(tc.tile_pool(name="wo_pool", bufs=1))
            opool = s3.enter_context(tc.tile_pool(name="outw", bufs=3))
            ps3 = s3.enter_context(tc.tile_pool(name="ps3", bufs=4, space="PSUM"))

            wo_sb = wpool.tile([P, HDT, D], f16, name="wo_sb")
            nc.sync.dma_start(out=wo_sb,
                              in_=t["Wo"].rearrange("(t p) n -> p t n", p=P))
            qres = wpool.tile([P, LQT, D], f32, name="qres")
            nc.sync.dma_start(out=qres,
                              in_=t["Qh"].rearrange("(t p) d -> p t d", p=P))
            bo_b = wpool.tile([P, D], f32, name="bo_b")
            nc.sync.dma_start(out=bo_b, in_=bcast_from_dram(t["bo"]))
            gam_b = wpool.tile([P, D], f32, name="gam_b")
            nc.sync.dma_start(out=gam_b, in_=bcast_from_dram(t["gamma"]))
            bet_b = wpool.tile([P, D], f32, name="bet_b")
            nc.sync.dma_start(out=bet_b, in_=bcast_from_dram(t["beta"]))
            eps_t = wpool.tile([P, 1], f32, name="eps_t")
            nc.vector.memset(eps_t, EPS)

            CHO = min(512, D)
            SG = D // CHO  # bn_stats subgroups
            out_dst = t["out_h"].rearrange("(t p) d -> p t d", p=P)
            for lt in range(LQT):
                s_t = opool.tile([P, D], f32, tag="s_t")
                for ci in range(D // CHO):
                    ps = ps3.tile([P, CHO], f32, tag="pso")
                    for dt_ in range(HDT):
                        nc.tensor.matmul(
                            ps,
                            lhsT=ctx_cat[:, dt_, lt * P:(lt + 1) * P],
                            rhs=wo_sb[:, dt_, ci * CHO:(ci + 1) * CHO],
                            start=(dt_ == 0), stop=(dt_ == HDT - 1))
                        # residual + bias
                    csl = slice(ci * CHO, (ci + 1) * CHO)
                    nc.vector.tensor_add(out=s_t[:, csl], in0=ps,
                                         in1=qres[:, lt, csl])
                nc.vector.tensor_add(out=s_t, in0=s_t, in1=bo_b)
                stats = opool.tile([P, SG, 6], f32, tag="stats")
                for i in range(SG):
                    nc.vector.bn_stats(out=stats[:, i, :],
                                       in_=s_t[:, i * CHO:(i + 1) * CHO])
                mv = opool.tile([P, 2], f32, tag="mv")
                nc.vector.bn_aggr(out=mv, in_=stats)
                std = opool.tile([P, 1], f32, tag="std")
                nc.scalar.activation(out=std, in_=mv[:, 1:2], func=Sqrt,
                                     bias=eps_t, scale=1.0)
                rstd = opool.tile([P, 1], f32, tag="rstd")
                nc.vector.reciprocal(out=rstd, in_=std)
                o_t = opool.tile([P, D], f32, tag="o_t")
                nc.vector.tensor_scalar(
                    out=o_t, in0=s_t, scalar1=mv[:, 0:1], scalar2=rstd,
                    op0=mybir.AluOpType.subtract, op1=mybir.AluOpType.mult)
                nc.vector.tensor_mul(out=o_t, in0=o_t, in1=gam_b)
                nc.vector.tensor_add(out=o_t, in0=o_t, in1=bet_b)
                nc.sync.dma_start(out=out_dst[:, lt, :], in_=o_t)

    nc.compile()
    return nc


def make_core_inputs(cfg, Q, K, V, attn_mask, Wq, bq, Wk, bk, Wv, bv, Wo, bo,
                     gamma, beta):
    """Host-side shard prep. Returns list of 8 input dicts (core order)."""
    P = 128
    LQ = cfg["LQ"]
    w_b = {n: np.asarray(w, np.float32).astype(F16)
           for n, w in (("Wq", Wq), ("Wk", Wk), ("Wv", Wv), ("Wo", Wo))}
    vecs = {n: np.ascontiguousarray(np.asarray(v, np.float32))
            for n, v in (("bq", bq), ("bk", bk), ("bv", bv), ("bo", bo),
                         ("gamma", gamma), ("beta", beta))}
    ident = np.eye(P, dtype=np.float32).astype(F8E5)
    B = Q.shape[0]
    per_b = {}
    for b in range(B):
        per_b[b] = dict(
            KT_in=np.ascontiguousarray(
                np.asarray(K[b], np.float32).T).astype(F16),
            VT_in=np.ascontiguousarray(
                np.asarray(V[b], np.float32).T).astype(F16),
        )
    in_maps = []
    n_halves = Q.shape[1] // LQ
    for c in range(B * n_halves):
        b, hf = divmod(c, n_halves)
        qs = slice(hf * LQ, (hf + 1) * LQ)
        Qh = np.ascontiguousarray(np.asarray(Q[b, qs], np.float32))
        QT_in = np.ascontiguousarray(Qh.T).astype(F16)
        m = np.asarray(attn_mask[b, qs], bool)
        maskT = np.where(m.T, np.float32(MASK_NEG),
                         np.float32(0.0)).astype(F8E5)
        in_maps.append(dict(
            QT_in=QT_in, maskT=maskT, ident=ident, Qh=Qh,
            **per_b[b], **w_b, **vecs))
    return in_maps


_PROGRAM_CACHE = {}


def _get_program():
    key = "full"
    if key not in _PROGRAM_CACHE:
        _PROGRAM_CACHE[key] = build_program(FULL_CFG)
    return _PROGRAM_CACHE[key]


def run_on_hw(inputs, trace=False, **kw):
    """Run the full-size kernel on the 8 NeuronCores. Returns BassKernelResults."""
    from concourse.bass_utils import run_bass_kernel_spmd
    nc = _get_program()
    in_maps = make_core_inputs(FULL_CFG, **inputs)
    return run_bass_kernel_spmd(nc, in_maps, core_ids=list(range(N_CORES)),
                                trace=trace, **kw)


def kernel(Q, K, V, attn_mask, Wq, bq, Wk, bk, Wv, bv, Wo, bo, gamma, beta):
    inputs = dict(Q=np.asarray(Q), K=np.asarray(K), V=np.asarray(V),
                  attn_mask=np.asarray(attn_mask), Wq=Wq, bq=bq, Wk=Wk, bk=bk,
                  Wv=Wv, bv=bv, Wo=Wo, bo=bo, gamma=gamma, beta=beta)
    res = run_on_hw(inputs).results
    B, L, D, H = B_FULL, L_FULL, D_FULL, H_FULL
    LQ = FULL_CFG["LQ"]
    out = np.empty((B, L, D), np.float32)
    attn = np.empty((B, H, L, L), np.float32)
    n_halves = L // LQ
    for c in range(N_CORES):
        b, hf = divmod(c, n_halves)
        qs = slice(hf * LQ, (hf + 1) * LQ)
        out[b, qs] = res[c]["out_h"]
        attn[b, :, qs, :] = res[c]["attn_t"].transpose(0, 2, 1)
    return out, attn


# revision 18
# speedup vs baseline: 1.4307x; 1.1324x over previous
"""Trainium2 Bass kernel for nn_MultiHeadAttention_89120571392727.

Reference computation (fp32):
    q = (Q @ Wq + bq)  -> [B, H, L, dk]   (per-head split)
    k, v likewise
    scores = q @ k^T / sqrt(dk);  scores = where(mask, -1e9, scores)
    attn = softmax(scores); context = attn @ v
    out = LayerNorm(Q + context_cat @ Wo + bo) * gamma + beta
    returns (out [B,L,D], attn [B,H,L,L])

Sharding (8 cores, no collectives): core c handles batch b=c//2 and
query-row half hf=c%2 (1024 rows), for ALL 16 heads.  Each core:
  - projections (bf16 matmuls, fp32 accumulate)
  - scores computed TRANSPOSED (keys on partitions, queries on free axis)
  - additive mask folded in via a PE identity-matmul accumulating an
    fp8 mask (0 / -57344) into the scores PSUM
  - exp on ScalarE with scale=1/8 (=1/sqrt(dk)); softmax denominators
    ride along the context matmul as a prepended ones-column of v
  - normalization on VectorE; attn written transposed [H, Lk, Lq]
    (host fixes the layout); out-projection + residual + LayerNorm on
    device.
Host work is limited to shard slicing / transposition / dtype casts of
inputs and reassembly of outputs.
"""

import os
import sys
from contextlib import ExitStack

import numpy as np

for _p in ("/opt/trn_rl_repo", "/opt/pypackages"):
    if _p not in sys.path and os.path.isdir(_p):
        sys.path.append(_p)

import ml_dtypes  # noqa: E402

F16 = np.float16
F8E5 = ml_dtypes.float8_e5m2
MASK_NEG = -57344.0  # most-negative finite f8e5m2; exp(-57344/8) == 0 in fp32

# Full-problem dimensions (hardcoded per the harness contract).
FULL_CFG = dict(
    D=1024,      # model dim
    H=16,        # heads
    DK=64,       # head dim
    LQ=1024,     # query rows per core (2048 / 2 halves)
    LK=2048,     # key rows
    QC=512,      # query-column chunk (<= 512, PSUM bank)
    KG=4,        # key-tile group size for probability staging
    EPS=1e-5,
)
N_CORES = 8
B_FULL, L_FULL, D_FULL, H_FULL = 4, 2048, 1024, 16

# Engine that casts fp32 probabilities to bf16 for the context matmul.
CAST_ENGINE = "gpsimd"  # or "vector"


def build_program(cfg, num_devices=N_CORES):
    """Build + schedule + compile the (SPMD, per-core) Bass program."""
    from concourse import bacc, mybir
    import concourse.tile as tile

    P = 128
    D, H, DK, LQ, LK, QC = (cfg[k] for k in ("D", "H", "DK", "LQ", "LK", "QC"))
    KG, EPS = cfg["KG"], cfg["EPS"]
    HD = H * DK
    DT, HDT, KT, LQT = D // P, HD // P, LK // P, LQ // P
    NQC = LQ // QC
    NG = KT // KG
    AW = DK + 1  # per-head width in v_aug (ones column first)
    assert D % P == 0 and HD % P == 0 and LK % P == 0 and LQ % P == 0
    assert QC <= 512 and LQ % QC == 0 and KT % KG == 0 and DK == 64

    f32 = mybir.dt.float32
    f16 = mybir.dt.float16
    f8 = mybir.dt.float8e5

    nc = bacc.Bacc("TRN2", target_bir_lowering=False, debug=False,
                   num_devices=num_devices)

    def din(name, shape, dtype):
        return nc.dram_tensor(name, list(shape), dtype, kind="ExternalInput").ap()

    def dout(name, shape, dtype):
        return nc.dram_tensor(name, list(shape), dtype, kind="ExternalOutput").ap()

    t = dict(
        QT_in=din("QT_in", (D, LQ), f16),
        KT_in=din("KT_in", (D, LK), f16),
        VT_in=din("VT_in", (D, LK), f16),
        maskT=din("maskT", (LK, LQ), f8),
        ident=din("ident", (P, P), f8),
        Wq=din("Wq", (D, HD), f16),
        Wk=din("Wk", (D, HD), f16),
        Wv=din("Wv", (D, HD), f16),
        Wo=din("Wo", (HD, D), f16),
        bq=din("bq", (HD,), f32),
        bk=din("bk", (HD,), f32),
        bv=din("bv", (HD,), f32),
        bo=din("bo", (D,), f32),
        gamma=din("gamma", (D,), f32),
        beta=din("beta", (D,), f32),
        Qh=din("Qh", (LQ, D), f32),
        out_h=dout("out_h", (LQ, D), f32),
        attn_t=dout("attn_t", (H, LK, LQ), f32),
    )

    import concourse.bass as bass

    def bcast_from_dram(vec_ap, parts=P):
        """[N] dram vector -> broadcast-read AP [parts, N] (partition step 0)."""
        return bass.AP(tensor=vec_ap.tensor, offset=vec_ap.offset,
                       ap=[[0, parts]] + [list(d) for d in vec_ap.ap])

    with tile.TileContext(nc) as tc, ExitStack() as ctx:
        Copy = mybir.ActivationFunctionType.Copy
        Identity = mybir.ActivationFunctionType.Identity
        Exp = mybir.ActivationFunctionType.Exp
        Sqrt = mybir.ActivationFunctionType.Sqrt

        persist = ctx.enter_context(tc.tile_pool(name="persist", bufs=1))
        qT = persist.tile([P, HDT, LQ], f16, name="qT")
        kT = persist.tile([P, HDT, LK], f16, name="kT")
        v_aug = persist.tile([P, KT, H * AW], f16, name="v_aug")
        ctx_cat = persist.tile([P, HDT, LQ], f16, name="ctx_cat")
        ident_sb = persist.tile([P, P], f8, name="ident_sb")
        nc.sync.dma_start(out=ident_sb, in_=t["ident"])

        # ---------------- stage 1: projections ----------------
        with ExitStack() as s1:
            xw = s1.enter_context(tc.tile_pool(name="xw", bufs=2))
            xin = s1.enter_context(tc.tile_pool(name="xin", bufs=1))
            bpool = s1.enter_context(tc.tile_pool(name="bias1", bufs=1))
            ps1 = s1.enter_context(tc.tile_pool(name="ps1", bufs=4, space="PSUM"))

            bq_sb = bpool.tile([P, HDT], f32, name="bq_sb")
            nc.sync.dma_start(out=bq_sb, in_=t["bq"].rearrange("(t p) -> p t", p=P))
            bk_sb = bpool.tile([P, HDT], f32, name="bk_sb")
            nc.sync.dma_start(out=bk_sb, in_=t["bk"].rearrange("(t p) -> p t", p=P))
            bv_sb = bpool.tile([P, HD], f32, name="bv_sb")
            nc.sync.dma_start(out=bv_sb, in_=bcast_from_dram(t["bv"]))

            # q / k projections produce transposed activations
            # xT_out[d', l] = sum_D W[D, d'] * XT_in[D, l]  (+ bias[d'])
            def proj_T(w_name, b_sb, in_name, out_tile, L):
                w_sb = xw.tile([P, DT, HD], f16, tag="w")
                nc.sync.dma_start(
                    out=w_sb, in_=t[w_name].rearrange("(t p) n -> p t n", p=P))
                x_sb = xin.tile([P, DT, L], f16, tag=f"xin{L}")
                nc.sync.dma_start(
                    out=x_sb, in_=t[in_name].rearrange("(t p) l -> p t l", p=P))
                CH = min(512, L)
                for ot in range(HDT):
                    for ci in range(L // CH):
                        ps = ps1.tile([P, CH], f32, tag="ps1")
                        for dt_ in range(DT):
                            nc.tensor.matmul(
                                ps,
                                lhsT=w_sb[:, dt_, ot * P:(ot + 1) * P],
                                rhs=x_sb[:, dt_, ci * CH:(ci + 1) * CH],
                                start=(dt_ == 0), stop=(dt_ == DT - 1))
                        nc.scalar.activation(
                            out=out_tile[:, ot, ci * CH:(ci + 1) * CH], in_=ps,
                            func=Identity, bias=b_sb[:, ot:ot + 1], scale=1.0)

            proj_T("Wq", bq_sb, "QT_in", qT, LQ)
            proj_T("Wk", bk_sb, "KT_in", kT, LK)

            # v projection in natural orientation: v[l, d'] (+ ones column
            # per head, prepended -> softmax denominators ride the matmul)
            w_sb = xw.tile([P, DT, HD], f16, tag="w")
            nc.sync.dma_start(
                out=w_sb, in_=t["Wv"].rearrange("(t p) n -> p t n", p=P))
            x_sb = xin.tile([P, DT, LK], f16, tag=f"xin{LK}")
            nc.sync.dma_start(
                out=x_sb, in_=t["VT_in"].rearrange("(t p) l -> p t l", p=P))
            v_by_head = v_aug.rearrange("p k (h w) -> p k h w", w=AW)
            nc.vector.memset(v_by_head[:, :, :, DK:DK + 1], 1.0)
            CHV = min(512, HD)
            HPC = CHV // DK  # heads per chunk
            for lt in range(KT):
                for ci in range(HD // CHV):
                    ps = ps1.tile([P, CHV], f32, tag="ps1")
                    for dt_ in range(DT):
                        nc.tensor.matmul(
                            ps,
                            lhsT=x_sb[:, dt_, lt * P:(lt + 1) * P],
                            rhs=w_sb[:, dt_, ci * CHV:(ci + 1) * CHV],
                            start=(dt_ == 0), stop=(dt_ == DT - 1))
                    nc.vector.tensor_add(
                        out=v_by_head[:, lt, ci * HPC:(ci + 1) * HPC, 0:DK],
                        in0=ps.rearrange("p (h d) -> p h d", d=DK),
                        in1=bv_sb.rearrange("p (h d) -> p h d", d=DK)[
                            :, ci * HPC:(ci + 1) * HPC, :])

        # ---------------- stage 2: attention ----------------
        with ExitStack() as s2:
            mpool = s2.enter_context(tc.tile_pool(name="maskp", bufs=2))
            ppool = s2.enter_context(tc.tile_pool(name="probs", bufs=2 * NG + 2))
            a4pool = s2.enter_context(tc.tile_pool(name="attn_st", bufs=4))
            rpool = s2.enter_context(tc.tile_pool(name="rvec", bufs=3))
            stpool = s2.enter_context(tc.tile_pool(name="ctxstage", bufs=2))
            ps_sc = s2.enter_context(tc.tile_pool(name="ps_sc", bufs=4, space="PSUM"))
            ps_cx = s2.enter_context(tc.tile_pool(name="ps_cx", bufs=3, space="PSUM"))
            ps_rb = s2.enter_context(tc.tile_pool(name="ps_rb", bufs=1, space="PSUM"))

            ones_c = rpool.tile([DK + 1, P], f32, name="ones_c", bufs=1)
            nc.vector.memset(ones_c[DK:DK + 1, :], 1.0)

            maskT_dram = t["maskT"].rearrange("(t p) q -> p t q", p=P)
            for qi in range(NQC):
                qsl = slice(qi * QC, (qi + 1) * QC)
                mT = mpool.tile([P, KT, QC], f8, tag="mT")
                nc.sync.dma_start(out=mT, in_=maskT_dram[:, :, qsl])
                # heads processed in (even, odd) pairs: their K=64 score
                # matmuls target disjoint PE row groups (base partitions 0
                # and 64) and execute concurrently when adjacent
                for hp in range(H // 2):
                    hh = (2 * hp, 2 * hp + 1)
                    ps_cs = [ps_cx.tile([AW, QC], f32, tag="psc", name="ps_c") for _ in hh]
                    p4s = ([], [])
                    # pass 1: scores + mask + exp (context deferred so the
                    # PE never stalls waiting on ScalarE's exp)
                    for g in range(NG):
                        P4p = [ppool.tile([P, KG, QC], f16, tag="p4", name="P4")
                               for _ in hh]
                        for j in range(KG):
                            kt = g * KG + j
                            ps_p = [ps_sc.tile([P, QC], f32, tag="pss", name="ps_s")
                                    for _ in hh]
                            for i, h in enumerate(hh):
                                po = (h % 2) * 64
                                nc.tensor.matmul(
                                    ps_p[i],
                                    lhsT=kT[po:po + DK, h // 2,
                                            kt * P:(kt + 1) * P],
                                    rhs=qT[po:po + DK, h // 2, qsl],
                                    start=True, stop=False)
                            for i in range(2):
                                nc.tensor.matmul(
                                    ps_p[i], lhsT=ident_sb, rhs=mT[:, kt, :],
                                    start=False, stop=True)
                            for i in range(2):
                                nc.scalar.activation(
                                    out=P4p[i][:, j, :], in_=ps_p[i],
                                    func=Exp, scale=0.125)
                        for i in range(2):
                            p4s[i].append(P4p[i])
                    # pass 2: context matmuls (sums ride as row DK via the
                    # ones column of v_aug)
                    for g in range(NG):
                        for j in range(KG):
                            kt = g * KG + j
                            for i, h in enumerate(hh):
                                nc.tensor.matmul(
                                    ps_cs[i],
                                    lhsT=v_aug[:, kt, h * AW:(h + 1) * AW],
                                    rhs=p4s[i][g][:, j, :],
                                    start=(kt == 0), stop=(kt == KT - 1))
                    # epilogue per head
                    for i, h in enumerate(hh):
                        po = (h % 2) * 64
                        ht = h // 2
                        ps_c = ps_cs[i]
                        r = rpool.tile([DK + 1, QC], f32, tag="r")
                        nc.vector.reciprocal(out=r[DK:DK + 1, :],
                                             in_=ps_c[DK:DK + 1, :])
                        rb = ps_rb.tile([P, QC], f32, tag="rb")
                        nc.tensor.matmul(rb, lhsT=ones_c[DK:DK + 1, :],
                                         rhs=r[DK:DK + 1, :],
                                         start=True, stop=True)
                        rb_sb = rpool.tile([P, QC], f32, tag="rb_sb")
                        nc.scalar.copy(out=rb_sb, in_=rb)
                        att_dst = t["attn_t"][h].rearrange(
                            "(t p) q -> p t q", p=P)
                        rb_b = rb_sb[:, None, :].to_broadcast((P, KG, QC))
                        for g, P4 in enumerate(p4s[i]):
                            A4 = a4pool.tile([P, KG, QC], f32, tag="a4")
                            nc.vector.tensor_mul(out=A4, in0=P4, in1=rb_b)
                            nc.sync.dma_start(
                                out=att_dst[:, g * KG:(g + 1) * KG, qsl],
                                in_=A4)
                        # context rows live on psum partitions 0..63;
                        # odd heads need a partition shift (ACT copy + DMA)
                        if po == 0:
                            nc.vector.tensor_mul(
                                out=ctx_cat[0:DK, ht, qsl],
                                in0=ps_c[0:DK, :], in1=rb_sb[0:DK, :])
                        else:
                            ctmp = stpool.tile([DK, QC], f32, tag="ctmp")
                            nc.scalar.activation(out=ctmp, in_=ps_c[0:DK, :],
                                                 func=Copy)
                            stg = stpool.tile([P, QC], f32, tag="stg")
                            nc.sync.dma_start(out=stg[po:po + DK, :], in_=ctmp)
                            nc.vector.tensor_mul(
                                out=ctx_cat[po:po + DK, ht, qsl],
                                in0=stg[po:po + DK, :],
                                in1=rb_sb[po:po + DK, :])

        # ---------------- stage 3: out-projection + LayerNorm ----------------
        with ExitStack() as s3:
            wpool = s3.enter_context(tc.tile_pool(name="wo_pool", bufs=1))
            opool = s3.enter_context(tc.tile_pool(name="outw", bufs=3))
            ps3 = s3.enter_context(tc.tile_pool(name="ps3", bufs=4, space="PSUM"))

            wo_sb = wpool.tile([P, HDT, D], f16, name="wo_sb")
            nc.sync.dma_start(out=wo_sb,
                              in_=t["Wo"].rearrange("(t p) n -> p t n", p=P))
            qres = wpool.tile([P, LQT, D], f32, name="qres")
            nc.sync.dma_start(out=qres,
                              in_=t["Qh"].rearrange("(t p) d -> p t d", p=P))
            bo_b = wpool.tile([P, D], f32, name="bo_b")
            nc.sync.dma_start(out=bo_b, in_=bcast_from_dram(t["bo"]))
            gam_b = wpool.tile([P, D], f32, name="gam_b")
            nc.sync.dma_start(out=gam_b, in_=bcast_from_dram(t["gamma"]))
            bet_b = wpool.tile([P, D], f32, name="bet_b")
            nc.sync.dma_start(out=bet_b, in_=bcast_from_dram(t["beta"]))
            eps_t = wpool.tile([P, 1], f32, name="eps_t")
            nc.vector.memset(eps_t, EPS)

            CHO = min(512, D)
            SG = D // CHO  # bn_stats subgroups
            out_dst = t["out_h"].rearrange("(t p) d -> p t d", p=P)
            for lt in range(LQT):
                s_t = opool.tile([P, D], f32, tag="s_t")
                for ci in range(D // CHO):
                    ps = ps3.tile([P, CHO], f32, tag="pso")
                    for dt_ in range(HDT):
                        nc.tensor.matmul(
                            ps,
                            lhsT=ctx_cat[:, dt_, lt * P:(lt + 1) * P],
                            rhs=wo_sb[:, dt_, ci * CHO:(ci + 1) * CHO],
                            start=(dt_ == 0), stop=(dt_ == HDT - 1))
                        # residual + bias
                    csl = slice(ci * CHO, (ci + 1) * CHO)
                    nc.vector.tensor_add(out=s_t[:, csl], in0=ps,
                                         in1=qres[:, lt, csl])
                nc.vector.tensor_add(out=s_t, in0=s_t, in1=bo_b)
                stats = opool.tile([P, SG, 6], f32, tag="stats")
                for i in range(SG):
                    nc.vector.bn_stats(out=stats[:, i, :],
                                       in_=s_t[:, i * CHO:(i + 1) * CHO])
                mv = opool.tile([P, 2], f32, tag="mv")
                nc.vector.bn_aggr(out=mv, in_=stats)
                std = opool.tile([P, 1], f32, tag="std")
                nc.scalar.activation(out=std, in_=mv[:, 1:2], func=Sqrt,
                                     bias=eps_t, scale=1.0)
                rstd = opool.tile([P, 1], f32, tag="rstd")
                nc.vector.reciprocal(out=rstd, in_=std)
                o_t = opool.tile([P, D], f32, tag="o_t")
                nc.vector.tensor_scalar(
                    out=o_t, in0=s_t, scalar1=mv[:, 0:1], scalar2=rstd,
                    op0=mybir.AluOpType.subtract, op1=mybir.AluOpType.mult)
                nc.vector.tensor_mul(out=o_t, in0=o_t, in1=gam_b)
                nc.vector.tensor_add(out=o_t, in0=o_t, in1=bet_b)
                nc.sync.dma_start(out=out_dst[:, lt, :], in_=o_t)

    nc.compile()
    return nc


def make_core_inputs(cfg, Q, K, V, attn_mask, Wq, bq, Wk, bk, Wv, bv, Wo, bo,
                     gamma, beta):
    """Host-side shard prep. Returns list of 8 input dicts (core order)."""
    P = 128
    LQ = cfg["LQ"]
    w_b = {n: np.asarray(w, np.float32).astype(F16)
           for n, w in (("Wq", Wq), ("Wk", Wk), ("Wv", Wv), ("Wo", Wo))}
    vecs = {n: np.ascontiguousarray(np.asarray(v, np.float32))
            for n, v in (("bq", bq), ("bk", bk), ("bv", bv), ("bo", bo),
                         ("gamma", gamma), ("beta", beta))}
    ident = np.eye(P, dtype=np.float32).astype(F8E5)
    B = Q.shape[0]
    per_b = {}
    for b in range(B):
        per_b[b] = dict(
            KT_in=np.ascontiguousarray(
                np.asarray(K[b], np.float32).T).astype(F16),
            VT_in=np.ascontiguousarray(
                np.asarray(V[b], np.float32).T).astype(F16),
        )
    in_maps = []
    n_halves = Q.shape[1] // LQ
    for c in range(B * n_halves):
        b, hf = divmod(c, n_halves)
        qs = slice(hf * LQ, (hf + 1) * LQ)
        Qh = np.ascontiguousarray(np.asarray(Q[b, qs], np.float32))
        QT_in = np.ascontiguousarray(Qh.T).astype(F16)
        m = np.asarray(attn_mask[b, qs], bool)
        maskT = np.where(m.T, np.float32(MASK_NEG),
                         np.float32(0.0)).astype(F8E5)
        in_maps.append(dict(
            QT_in=QT_in, maskT=maskT, ident=ident, Qh=Qh,
            **per_b[b], **w_b, **vecs))
    return in_maps


_PROGRAM_CACHE = {}


def _get_program():
    key = "full"
    if key not in _PROGRAM_CACHE:
        _PROGRAM_CACHE[key] = build_program(FULL_CFG)
    return _PROGRAM_CACHE[key]


def run_on_hw(inputs, trace=False, **kw):
    """Run the full-size kernel on the 8 NeuronCores. Returns BassKernelResults."""
    from concourse.bass_utils import run_bass_kernel_spmd
    nc = _get_program()
    in_maps = make_core_inputs(FULL_CFG, **inputs)
    return run_bass_kernel_spmd(nc, in_maps, core_ids=list(range(N_CORES)),
                                trace=trace, **kw)


def kernel(Q, K, V, attn_mask, Wq, bq, Wk, bk, Wv, bv, Wo, bo, gamma, beta):
    inputs = dict(Q=np.asarray(Q), K=np.asarray(K), V=np.asarray(V),
                  attn_mask=np.asarray(attn_mask), Wq=Wq, bq=bq, Wk=Wk, bk=bk,
                  Wv=Wv, bv=bv, Wo=Wo, bo=bo, gamma=gamma, beta=beta)
    res = run_on_hw(inputs).results
    B, L, D, H = B_FULL, L_FULL, D_FULL, H_FULL
    LQ = FULL_CFG["LQ"]
    out = np.empty((B, L, D), np.float32)
    attn = np.empty((B, H, L, L), np.float32)
    n_halves = L // LQ
    for c in range(N_CORES):
        b, hf = divmod(c, n_halves)
        qs = slice(hf * LQ, (hf + 1) * LQ)
        out[b, qs] = res[c]["out_h"]
        attn[b, :, qs, :] = res[c]["attn_t"].transpose(0, 2, 1)
    return out, attn


# revision 24
# speedup vs baseline: 1.6863x; 1.1786x over previous
"""Trainium2 Bass kernel for nn_MultiHeadAttention_89120571392727.

Reference computation (fp32):
    q = (Q @ Wq + bq)  -> [B, H, L, dk]   (per-head split)
    k, v likewise
    scores = q @ k^T / sqrt(dk);  scores = where(mask, -1e9, scores)
    attn = softmax(scores); context = attn @ v
    out = LayerNorm(Q + context_cat @ Wo + bo) * gamma + beta
    returns (out [B,L,D], attn [B,H,L,L])

Sharding (8 cores, no collectives): core c handles batch b=c//2 and
query-row half hf=c%2 (1024 rows), for ALL 16 heads.  Each core:
  - projections (bf16 matmuls, fp32 accumulate)
  - scores computed TRANSPOSED (keys on partitions, queries on free axis)
  - additive mask folded in via a PE identity-matmul accumulating an
    fp8 mask (0 / -57344) into the scores PSUM
  - exp on ScalarE with scale=1/8 (=1/sqrt(dk)); softmax denominators
    ride along the context matmul as a prepended ones-column of v
  - normalization on VectorE; attn written transposed [H, Lk, Lq]
    (host fixes the layout); out-projection + residual + LayerNorm on
    device.
Host work is limited to shard slicing / transposition / dtype casts of
inputs and reassembly of outputs.
"""

import os
import sys
from contextlib import ExitStack

import numpy as np

for _p in ("/opt/trn_rl_repo", "/opt/pypackages"):
    if _p not in sys.path and os.path.isdir(_p):
        sys.path.append(_p)

import ml_dtypes  # noqa: E402

F16 = np.float16
F8E5 = ml_dtypes.float8_e5m2
MASK_NEG = -57344.0  # most-negative finite f8e5m2; exp(-57344/8) == 0 in fp32

# Full-problem dimensions (hardcoded per the harness contract).
FULL_CFG = dict(
    D=1024,      # model dim
    H=16,        # heads
    DK=64,       # head dim
    LQ=1024,     # query rows per core (2048 / 2 halves)
    LK=2048,     # key rows
    QC=512,      # query-column chunk (<= 512, PSUM bank)
    KG=4,        # key-tile group size for probability staging
    EPS=1e-5,
)
N_CORES = 8
B_FULL, L_FULL, D_FULL, H_FULL = 4, 2048, 1024, 16

# Engine that casts fp32 probabilities to bf16 for the context matmul.
CAST_ENGINE = "gpsimd"  # or "vector"


def build_program(cfg, num_devices=N_CORES):
    """Build + schedule + compile the (SPMD, per-core) Bass program."""
    from concourse import bacc, mybir
    import concourse.tile as tile

    P = 128
    D, H, DK, LQ, LK, QC = (cfg[k] for k in ("D", "H", "DK", "LQ", "LK", "QC"))
    KG, EPS = cfg["KG"], cfg["EPS"]
    HD = H * DK
    DT, HDT, KT, LQT = D // P, HD // P, LK // P, LQ // P
    NQC = LQ // QC
    NG = KT // KG
    AW = DK + 1  # per-head width in v_aug (ones column first)
    assert D % P == 0 and HD % P == 0 and LK % P == 0 and LQ % P == 0
    assert QC <= 512 and LQ % QC == 0 and KT % KG == 0 and DK == 64

    f32 = mybir.dt.float32
    f16 = mybir.dt.float16
    f8 = mybir.dt.float8e5

    nc = bacc.Bacc("TRN2", target_bir_lowering=False, debug=False,
                   num_devices=num_devices)

    def din(name, shape, dtype):
        return nc.dram_tensor(name, list(shape), dtype, kind="ExternalInput").ap()

    def dout(name, shape, dtype):
        return nc.dram_tensor(name, list(shape), dtype, kind="ExternalOutput").ap()

    t = dict(
        QT_in=din("QT_in", (D, LQ), f16),
        KT_in=din("KT_in", (D, LK), f16),
        VT_in=din("VT_in", (D, LK), f16),
        maskT=din("maskT", (LK, LQ), f8),
        ident=din("ident", (P, P), f8),
        Wq=din("Wq", (D, HD), f16),
        Wk=din("Wk", (D, HD), f16),
        Wv=din("Wv", (D, HD), f16),
        Wo=din("Wo", (HD, D), f16),
        bq=din("bq", (HD,), f32),
        bk=din("bk", (HD,), f32),
        bv=din("bv", (HD,), f32),
        bo=din("bo", (D,), f32),
        gamma=din("gamma", (D,), f32),
        beta=din("beta", (D,), f32),
        Qh=din("Qh", (LQ, D), f32),
        out_h=dout("out_h", (LQ, D), f32),
        attn_t=dout("attn_t", (H, LK, LQ), f32),
    )

    import concourse.bass as bass

    def bcast_from_dram(vec_ap, parts=P):
        """[N] dram vector -> broadcast-read AP [parts, N] (partition step 0)."""
        return bass.AP(tensor=vec_ap.tensor, offset=vec_ap.offset,
                       ap=[[0, parts]] + [list(d) for d in vec_ap.ap])

    with tile.TileContext(nc) as tc, ExitStack() as ctx:
        Copy = mybir.ActivationFunctionType.Copy
        Identity = mybir.ActivationFunctionType.Identity
        Exp = mybir.ActivationFunctionType.Exp
        Sqrt = mybir.ActivationFunctionType.Sqrt

        persist = ctx.enter_context(tc.tile_pool(name="persist", bufs=1))
        qT = persist.tile([P, HDT, LQ], f16, name="qT")
        kT = persist.tile([P, HDT, LK], f16, name="kT")
        v_aug = persist.tile([P, KT, H * AW], f16, name="v_aug")
        ctx_cat = persist.tile([P, HDT, LQ], f16, name="ctx_cat")
        ident_sb = persist.tile([P, P], f8, name="ident_sb")
        nc.sync.dma_start(out=ident_sb, in_=t["ident"])

        # ---------------- stage 1: projections ----------------
        with ExitStack() as s1:
            xw = s1.enter_context(tc.tile_pool(name="xw", bufs=2))
            xin = s1.enter_context(tc.tile_pool(name="xin", bufs=1))
            bpool = s1.enter_context(tc.tile_pool(name="bias1", bufs=1))
            ps1 = s1.enter_context(tc.tile_pool(name="ps1", bufs=4, space="PSUM"))

            bq_sb = bpool.tile([P, HDT], f32, name="bq_sb")
            nc.sync.dma_start(out=bq_sb, in_=t["bq"].rearrange("(t p) -> p t", p=P))
            bk_sb = bpool.tile([P, HDT], f32, name="bk_sb")
            nc.sync.dma_start(out=bk_sb, in_=t["bk"].rearrange("(t p) -> p t", p=P))
            bv_sb = bpool.tile([P, HD], f32, name="bv_sb")
            nc.sync.dma_start(out=bv_sb, in_=bcast_from_dram(t["bv"]))

            # q / k projections produce transposed activations
            # xT_out[d', l] = sum_D W[D, d'] * XT_in[D, l]  (+ bias[d'])
            def proj_T(w_name, b_sb, in_name, out_tile, L):
                w_sb = xw.tile([P, DT, HD], f16, tag="w")
                nc.sync.dma_start(
                    out=w_sb, in_=t[w_name].rearrange("(t p) n -> p t n", p=P))
                x_sb = xin.tile([P, DT, L], f16, tag=f"xin{L}")
                nc.sync.dma_start(
                    out=x_sb, in_=t[in_name].rearrange("(t p) l -> p t l", p=P))
                CH = min(512, L)
                for ot in range(HDT):
                    for ci in range(L // CH):
                        ps = ps1.tile([P, CH], f32, tag="ps1")
                        for dt_ in range(DT):
                            nc.tensor.matmul(
                                ps,
                                lhsT=w_sb[:, dt_, ot * P:(ot + 1) * P],
                                rhs=x_sb[:, dt_, ci * CH:(ci + 1) * CH],
                                start=(dt_ == 0), stop=(dt_ == DT - 1))
                        nc.scalar.activation(
                            out=out_tile[:, ot, ci * CH:(ci + 1) * CH], in_=ps,
                            func=Identity, bias=b_sb[:, ot:ot + 1], scale=1.0)

            proj_T("Wq", bq_sb, "QT_in", qT, LQ)
            proj_T("Wk", bk_sb, "KT_in", kT, LK)

            # v projection in natural orientation: v[l, d'] (+ ones column
            # per head, prepended -> softmax denominators ride the matmul)
            w_sb = xw.tile([P, DT, HD], f16, tag="w")
            nc.sync.dma_start(
                out=w_sb, in_=t["Wv"].rearrange("(t p) n -> p t n", p=P))
            x_sb = xin.tile([P, DT, LK], f16, tag=f"xin{LK}")
            nc.sync.dma_start(
                out=x_sb, in_=t["VT_in"].rearrange("(t p) l -> p t l", p=P))
            v_by_head = v_aug.rearrange("p k (h w) -> p k h w", w=AW)
            nc.vector.memset(v_by_head[:, :, :, DK:DK + 1], 1.0)
            CHV = min(512, HD)
            HPC = CHV // DK  # heads per chunk
            for lt in range(KT):
                for ci in range(HD // CHV):
                    ps = ps1.tile([P, CHV], f32, tag="ps1")
                    for dt_ in range(DT):
                        nc.tensor.matmul(
                            ps,
                            lhsT=x_sb[:, dt_, lt * P:(lt + 1) * P],
                            rhs=w_sb[:, dt_, ci * CHV:(ci + 1) * CHV],
                            start=(dt_ == 0), stop=(dt_ == DT - 1))
                    nc.vector.tensor_add(
                        out=v_by_head[:, lt, ci * HPC:(ci + 1) * HPC, 0:DK],
                        in0=ps.rearrange("p (h d) -> p h d", d=DK),
                        in1=bv_sb.rearrange("p (h d) -> p h d", d=DK)[
                            :, ci * HPC:(ci + 1) * HPC, :])

        # ---------------- stage 2: attention ----------------
        with ExitStack() as s2:
            mpool = s2.enter_context(tc.tile_pool(name="maskp", bufs=2))
            ppool = s2.enter_context(tc.tile_pool(name="probs", bufs=3 * NG - 1))
            a4pool = s2.enter_context(tc.tile_pool(name="attn_st", bufs=3))
            rpool = s2.enter_context(tc.tile_pool(name="rvec", bufs=2))
            stpool = s2.enter_context(tc.tile_pool(name="ctxstage", bufs=2))
            ps_sc = s2.enter_context(tc.tile_pool(name="ps_sc", bufs=5, space="PSUM"))
            ps_cx = s2.enter_context(tc.tile_pool(name="ps_cx", bufs=3, space="PSUM"))

            maskT_dram = t["maskT"].rearrange("(t p) q -> p t q", p=P)
            for qi in range(NQC):
                qsl = slice(qi * QC, (qi + 1) * QC)
                mT = mpool.tile([P, KT, QC], f8, tag="mT")
                nc.sync.dma_start(out=mT, in_=maskT_dram[:, :, qsl])
                # heads processed in (even, odd) pairs: their K=64 score
                # matmuls target disjoint PE row groups (base partitions 0
                # and 64) and execute concurrently when adjacent
                for hp in range(H // 2):
                    hh = (2 * hp, 2 * hp + 1)
                    ps_cs = [ps_cx.tile([AW, QC], f32, tag="psc", name="ps_c") for _ in hh]
                    p4s = ([], [])
                    # pass 1: scores + mask + exp (context deferred so the
                    # PE never stalls waiting on ScalarE's exp)
                    for g in range(NG):
                        P4p = [ppool.tile([P, KG, QC], f16, tag="p4", name="P4")
                               for _ in hh]
                        for j in range(KG):
                            kt = g * KG + j
                            ps_p = [ps_sc.tile([P, QC], f32, tag="pss", name="ps_s")
                                    for _ in hh]
                            for i, h in enumerate(hh):
                                po = (h % 2) * 64
                                nc.tensor.matmul(
                                    ps_p[i],
                                    lhsT=kT[po:po + DK, h // 2,
                                            kt * P:(kt + 1) * P],
                                    rhs=qT[po:po + DK, h // 2, qsl],
                                    start=True, stop=False)
                            for i in range(2):
                                nc.tensor.matmul(
                                    ps_p[i], lhsT=ident_sb, rhs=mT[:, kt, :],
                                    start=False, stop=True)
                            for i in range(2):
                                nc.scalar.activation(
                                    out=P4p[i][:, j, :], in_=ps_p[i],
                                    func=Exp, scale=0.125)
                        for i in range(2):
                            p4s[i].append(P4p[i])
                    # pass 2: context matmuls (sums ride as row DK via the
                    # ones column of v_aug)
                    for g in range(NG):
                        for j in range(KG):
                            kt = g * KG + j
                            for i, h in enumerate(hh):
                                nc.tensor.matmul(
                                    ps_cs[i],
                                    lhsT=v_aug[:, kt, h * AW:(h + 1) * AW],
                                    rhs=p4s[i][g][:, j, :],
                                    start=(kt == 0), stop=(kt == KT - 1))
                    # epilogue per head
                    for i, h in enumerate(hh):
                        po = (h % 2) * 64
                        ht = h // 2
                        ps_c = ps_cs[i]
                        # denominator chain pinned to physical partition 0
                        # (custom DVE ops + partition_broadcast mishandle
                        # partition-based APs on hardware)
                        sm = rpool.tile([DK + 1, QC], f32, tag="sm", bufs=1)
                        nc.scalar.activation(out=sm[DK:DK + 1, :],
                                             in_=ps_c[DK:DK + 1, :], func=Copy)
                        s0 = rpool.tile([1, QC], f32, tag="s0", bufs=1)
                        nc.sync.dma_start(out=s0, in_=sm[DK:DK + 1, :])
                        r0 = rpool.tile([1, QC], f32, tag="r0", bufs=1)
                        rs = rpool.tile([1, QC], f32, tag="rs", bufs=1)
                        nc.vector.reciprocal_approx_accurate(
                            out=r0, in_=s0, scratch=rs)
                        rb_sb = rpool.tile([P, QC], f32, tag="rb_sb")
                        nc.gpsimd.partition_broadcast(rb_sb, r0)
                        att_dst = t["attn_t"][h].rearrange(
                            "(t p) q -> p t q", p=P)
                        rb_b = rb_sb[:, None, :].to_broadcast((P, KG, QC))
                        for g, P4 in enumerate(p4s[i]):
                            A4 = a4pool.tile([P, KG, QC], f32, tag="a4")
                            nc.vector.tensor_mul(out=A4, in0=P4, in1=rb_b)
                            nc.sync.dma_start(
                                out=att_dst[:, g * KG:(g + 1) * KG, qsl],
                                in_=A4)
                        # context rows live on psum partitions 0..63;
                        # odd heads need a partition shift (ACT copy + DMA)
                        if po == 0:
                            nc.vector.tensor_mul(
                                out=ctx_cat[0:DK, ht, qsl],
                                in0=ps_c[0:DK, :], in1=rb_sb[0:DK, :])
                        else:
                            ctmp = stpool.tile([DK, QC], f32, tag="ctmp")
                            nc.scalar.activation(out=ctmp, in_=ps_c[0:DK, :],
                                                 func=Copy)
                            stg = stpool.tile([P, QC], f32, tag="stg")
                            nc.sync.dma_start(out=stg[po:po + DK, :], in_=ctmp)
                            nc.vector.tensor_mul(
                                out=ctx_cat[po:po + DK, ht, qsl],
                                in0=stg[po:po + DK, :],
                                in1=rb_sb[po:po + DK, :])

        # ---------------- stage 3: out-projection + LayerNorm ----------------
        with ExitStack() as s3:
            wpool = s3.enter_context(tc.tile_pool(name="wo_pool", bufs=1))
            opool = s3.enter_context(tc.tile_pool(name="outw", bufs=3))
            ps3 = s3.enter_context(tc.tile_pool(name="ps3", bufs=4, space="PSUM"))

            wo_sb = wpool.tile([P, HDT, D], f16, name="wo_sb")
            nc.sync.dma_start(out=wo_sb,
                              in_=t["Wo"].rearrange("(t p) n -> p t n", p=P))
            qres = wpool.tile([P, LQT, D], f32, name="qres")
            nc.sync.dma_start(out=qres,
                              in_=t["Qh"].rearrange("(t p) d -> p t d", p=P))
            bo_b = wpool.tile([P, D], f32, name="bo_b")
            nc.sync.dma_start(out=bo_b, in_=bcast_from_dram(t["bo"]))
            gam_b = wpool.tile([P, D], f32, name="gam_b")
            nc.sync.dma_start(out=gam_b, in_=bcast_from_dram(t["gamma"]))
            bet_b = wpool.tile([P, D], f32, name="bet_b")
            nc.sync.dma_start(out=bet_b, in_=bcast_from_dram(t["beta"]))
            eps_t = wpool.tile([P, 1], f32, name="eps_t")
            nc.vector.memset(eps_t, EPS)

            CHO = min(512, D)
            SG = D // CHO  # bn_stats subgroups
            out_dst = t["out_h"].rearrange("(t p) d -> p t d", p=P)
            for lt in range(LQT):
                s_t = opool.tile([P, D], f32, tag="s_t")
                for ci in range(D // CHO):
                    ps = ps3.tile([P, CHO], f32, tag="pso")
                    for dt_ in range(HDT):
                        nc.tensor.matmul(
                            ps,
                            lhsT=ctx_cat[:, dt_, lt * P:(lt + 1) * P],
                            rhs=wo_sb[:, dt_, ci * CHO:(ci + 1) * CHO],
                            start=(dt_ == 0), stop=(dt_ == HDT - 1))
                        # residual + bias
                    csl = slice(ci * CHO, (ci + 1) * CHO)
                    nc.vector.tensor_add(out=s_t[:, csl], in0=ps,
                                         in1=qres[:, lt, csl])
                nc.vector.tensor_add(out=s_t, in0=s_t, in1=bo_b)
                stats = opool.tile([P, SG, 6], f32, tag="stats")
                for i in range(SG):
                    nc.vector.bn_stats(out=stats[:, i, :],
                                       in_=s_t[:, i * CHO:(i + 1) * CHO])
                mv = opool.tile([P, 2], f32, tag="mv")
                nc.vector.bn_aggr(out=mv, in_=stats)
                std = opool.tile([P, 1], f32, tag="std")
                nc.scalar.activation(out=std, in_=mv[:, 1:2], func=Sqrt,
                                     bias=eps_t, scale=1.0)
                rstd = opool.tile([P, 1], f32, tag="rstd")
                nc.vector.reciprocal(out=rstd, in_=std)
                o_t = opool.tile([P, D], f32, tag="o_t")
                nc.vector.tensor_scalar(
                    out=o_t, in0=s_t, scalar1=mv[:, 0:1], scalar2=rstd,
                    op0=mybir.AluOpType.subtract, op1=mybir.AluOpType.mult)
                nc.vector.tensor_mul(out=o_t, in0=o_t, in1=gam_b)
                nc.vector.tensor_add(out=o_t, in0=o_t, in1=bet_b)
                nc.sync.dma_start(out=out_dst[:, lt, :], in_=o_t)

    nc.compile()
    return nc


def make_core_inputs(cfg, Q, K, V, attn_mask, Wq, bq, Wk, bk, Wv, bv, Wo, bo,
                     gamma, beta):
    """Host-side shard prep. Returns list of 8 input dicts (core order)."""
    P = 128
    LQ = cfg["LQ"]
    w_b = {n: np.asarray(w, np.float32).astype(F16)
           for n, w in (("Wq", Wq), ("Wk", Wk), ("Wv", Wv), ("Wo", Wo))}
    vecs = {n: np.ascontiguousarray(np.asarray(v, np.float32))
            for n, v in (("bq", bq), ("bk", bk), ("bv", bv), ("bo", bo),
                         ("gamma", gamma), ("beta", beta))}
    ident = np.eye(P, dtype=np.float32).astype(F8E5)
    B = Q.shape[0]
    per_b = {}
    for b in range(B):
        per_b[b] = dict(
            KT_in=np.ascontiguousarray(
                np.asarray(K[b], np.float32).T).astype(F16),
            VT_in=np.ascontiguousarray(
                np.asarray(V[b], np.float32).T).astype(F16),
        )
    in_maps = []
    n_halves = Q.shape[1] // LQ
    for c in range(B * n_halves):
        b, hf = divmod(c, n_halves)
        qs = slice(hf * LQ, (hf + 1) * LQ)
        Qh = np.ascontiguousarray(np.asarray(Q[b, qs], np.float32))
        QT_in = np.ascontiguousarray(Qh.T).astype(F16)
        m = np.asarray(attn_mask[b, qs], bool)
        maskT = np.where(m.T, np.float32(MASK_NEG),
                         np.float32(0.0)).astype(F8E5)
        in_maps.append(dict(
            QT_in=QT_in, maskT=maskT, ident=ident, Qh=Qh,
            **per_b[b], **w_b, **vecs))
    return in_maps


_PROGRAM_CACHE = {}


def _get_program():
    key = "full"
    if key not in _PROGRAM_CACHE:
        _PROGRAM_CACHE[key] = build_program(FULL_CFG)
    return _PROGRAM_CACHE[key]


def run_on_hw(inputs, trace=False, **kw):
    """Run the full-size kernel on the 8 NeuronCores. Returns BassKernelResults."""
    from concourse.bass_utils import run_bass_kernel_spmd
    nc = _get_program()
    in_maps = make_core_inputs(FULL_CFG, **inputs)
    return run_bass_kernel_spmd(nc, in_maps, core_ids=list(range(N_CORES)),
                                trace=trace, **kw)


def kernel(Q, K, V, attn_mask, Wq, bq, Wk, bk, Wv, bv, Wo, bo, gamma, beta):
    inputs = dict(Q=np.asarray(Q), K=np.asarray(K), V=np.asarray(V),
                  attn_mask=np.asarray(attn_mask), Wq=Wq, bq=bq, Wk=Wk, bk=bk,
                  Wv=Wv, bv=bv, Wo=Wo, bo=bo, gamma=gamma, beta=beta)
    res = run_on_hw(inputs).results
    B, L, D, H = B_FULL, L_FULL, D_FULL, H_FULL
    LQ = FULL_CFG["LQ"]
    out = np.empty((B, L, D), np.float32)
    attn = np.empty((B, H, L, L), np.float32)
    n_halves = L // LQ
    for c in range(N_CORES):
        b, hf = divmod(c, n_halves)
        qs = slice(hf * LQ, (hf + 1) * LQ)
        out[b, qs] = res[c]["out_h"]
        attn[b, :, qs, :] = res[c]["attn_t"].transpose(0, 2, 1)
    return out, attn
